# revision 2
# baseline (speedup 1.0000x reference)
"""MoE grouped-GEMM (8 experts) on 8 Trainium2 NeuronCores.

Problem: input [32768, 1024] routed contiguously to 8 experts (counts in
num_experts_per_token); expert i computes x_i @ W_i.T + b_i with
W [8, 4096, 1024], b [8, 4096]. Output [32768, 4096].

Sharding: expert-parallel, expert i <-> core i, zero collectives. Host
slices each expert's token block and packs operands into exact SBUF tile
layouts; each core runs a 4096x1024x4096 GEMM; host adds bias (fp32) and
concatenates.

Per-core kernel, PE-bound. Mixed-precision contraction split (rel err
1.895e-2 measured end-to-end vs the 2e-2 gate):
  - k-tiles 2..7 (K=768): x bf16 STATIONARY x w bf16 MOVING, plain
    matmuls [K128, M128, N512] at ~216 ns.
  - k-tiles 0..1 (K=256): ONE fp8 e4m3 DoubleRow matmul per (m,n)
    (~0.53x the per-row cost of bf16). This bf16/fp8 mix is optimal on
    the error/speed frontier: full-K e4m3 measures 3.7e-2 (> the 2e-2
    gate), and HW probes show DoublePixel/DoubleColumn are ignored and
    e3m4+DoubleRow rejected on TRN2, so no cheaper mode exists.
  - Both parts accumulate into the SAME PSUM bank: operands pre-scaled
    per-expert so every partial product carries a*b = 2^16; the drain
    is a single x 2^-16 multiply fused with the PSUM->SBUF bf16 copy,
    alternating DVE/ACT. Host adds bias in fp32.

Startup (the one phase where HBM supply < PE demand): the first four
m-blocks run a k-progressive GROUP schedule — group = 4 m-blocks x
2 n-tiles = 8 PSUM banks, j advances only after all 8 pairs did j —
dropping early weight demand from ~578 GB/s (one block's j-loop) to
~145 GB/s, under the ~358 GB/s per-core HBM supply. The DGE rings are
~8-deep self-gated windows (trigger N+8 waits on desc N), so DMA issue
order IS priority order: w goes on the sync ring in exact consumption
order (q0 as 128 KB n-tiles so the first matmuls gate small, then w8
q0/q1, then q1..q3 quarters), x on the scalar ring (j-sliced for the
batch), phase-A output DMAs on scalar/gpsimd (the sync ring's window
would delay them and pin ot buffers). DR sections keep block-parity
adjacency (g0 ends DR, g1 starts DR, ...) so DR<->bf16 transitions
stay minimal. m-blocks 4.. run the steady-state loop (k-outer/n-inner,
stationary reused 8x so LDWEIGHTS hides, 2-ahead x prefetch); the last
block is n-outer/k-inner with a split final drain + split final y DMA
to shorten the tail. Measured 411.2-412.6 us vs 420.9 us for the
previous m-outer-only schedule.
"""

import sys

if "/opt/trn_rl_repo" not in sys.path:
    sys.path.insert(0, "/opt/trn_rl_repo")

import numpy as np

E, T, DIN, DOUT = 8, 32768, 1024, 4096
NCORES = 8
TOKC = T // NCORES  # tokens per core (capacity)

KT = 128   # contraction tile (SBUF partitions)
MT = 128   # token tile (PSUM partitions)
NT = 512   # dout tile (one fp32 PSUM bank)
KTILES = DIN // KT    # 8
MTILES = TOKC // MT   # 32
NTILES = DOUT // NT   # 8

KF8 = 256           # contraction cols computed in fp8 DoubleRow (k-tiles 0,1)
K16T = (DIN - KF8) // KT  # 6 bf16 k-tiles (2..7)
PSCALE = 2.0 ** 16  # fixed a*b product; drain multiplies by 1/PSCALE
F8MAX = 240.0       # float8_e4m3 max normal
F8MARGIN = 0.98

GB = 4              # m-blocks in the k-progressive startup batch
_CACHE = {}


def _build_nc():
    import concourse.bacc as bacc
    import concourse.tile as tile
    import concourse.mybir as mybir

    nc = bacc.Bacc("TRN2", target_bir_lowering=False, debug=False,
                   num_devices=NCORES)

    # x16B[m][kk, j*MT + t] = bf16(x[m*MT + t, (j+2)*KT + kk] * a)
    x16B = nc.dram_tensor("x16B", [MTILES, KT, K16T * MT], mybir.dt.bfloat16,
                          kind="ExternalInput")
    # x8B[m][kk, i*MT + t] = e4m3(x[m*MT + t, i*KT + kk] * a), i in {0,1}
    x8B = nc.dram_tensor("x8B", [MTILES, KT, 2 * MT], mybir.dt.float8e4,
                         kind="ExternalInput")
    # w16B[j][kk, d] = bf16(w[d, (j+2)*KT + kk] * b)
    w16B = nc.dram_tensor("w16B", [K16T, KT, DOUT], mybir.dt.bfloat16,
                          kind="ExternalInput")
    # w8B[kk, i*DOUT + d] = e4m3(w[d, i*KT + kk] * b)
    w8B = nc.dram_tensor("w8B", [KT, 2 * DOUT], mybir.dt.float8e4,
                         kind="ExternalInput")
    y = nc.dram_tensor("y", [TOKC, DOUT], mybir.dt.bfloat16,
                       kind="ExternalOutput")

    DR = mybir.MatmulPerfMode.DoubleRow
    INV = 1.0 / PSCALE
    Q = DOUT // 4   # w16 n-quarter (1024 cols, 256 KB bf16)

    with tile.TileContext(nc) as tc:
        with (
            tc.tile_pool(name="wpool", bufs=1) as wpool,
            tc.tile_pool(name="xpool", bufs=6) as xpool,
            tc.tile_pool(name="opool", bufs=12) as opool,
            tc.tile_pool(name="psum", bufs=8, space="PSUM") as psum_pool,
        ):
            wt = [wpool.tile([KT, DOUT], mybir.dt.bfloat16,
                             name=f"wt{j}", tag=f"wt{j}")
                  for j in range(K16T)]
            w8t = wpool.tile([KT, 2, DOUT], mybir.dt.float8e4,
                             name="w8t", tag="w8t")

            # ---- supply: deadline-ordered, all w on the sync ring ----
            # The DGE rings are ~8-deep self-gated windows (trigger N+8
            # waits on desc N), so issue order IS priority order. q0 is
            # sliced to n-tiles (128 KB) so the first MMs gate small.
            def wq(j, q):
                nc.sync.dma_start(wt[j][:, q * Q:(q + 1) * Q],
                                  w16B[j][:, q * Q:(q + 1) * Q])

            def w8q(q, eng=None):
                # both i-slots for n-quarter q
                for i in range(2):
                    (eng or nc.sync).dma_start(
                        w8t[:, i, q * Q:(q + 1) * Q],
                        w8B[:, i * DOUT + q * Q:i * DOUT + (q + 1) * Q])

            # consumption order: g0 bf16 q0 -> g0 DR (w8q0) -> g1 DR
            # (w8q1) -> g1 bf16 q1 -> g2 bf16 q2 -> g2 DR (w8q2) ->
            # g3 DR (w8q3) -> g3 bf16 q3. q1..q3 quarters alternate
            # sync/scalar so delivery keeps pace with the j-loop.
            for j in range(K16T):
                for n in range(2):
                    nc.sync.dma_start(wt[j][:, n * NT:(n + 1) * NT],
                                      w16B[j][:, n * NT:(n + 1) * NT])
            # w8 q0/q1 directly after q0 (deadlines: g0-end / g1-start);
            # q1 sliced to n-tiles for finer pacing against g1's j-loop.
            w8q(0)
            w8q(1)
            for j in range(K16T):
                wq(j, 1)
            for j in range(K16T):
                wq(j, 2)
            w8q(2)
            w8q(3)
            for j in range(K16T):
                wq(j, 3)

            # scalar HWDGE ring: x for the startup batch, j-sliced so
            # group j-levels gate on 64 KB slices; x8 after; then the
            # steady-state prefetch of m = GB, GB+1.
            x16t = [None] * MTILES
            x8t = [None] * MTILES

            def load_x16(m, sliced=False):
                t16 = xpool.tile([KT, K16T, MT], mybir.dt.bfloat16,
                                 name="xm16", tag="xm16")
                if sliced:
                    for j2 in range(0, K16T, 2):
                        nc.scalar.dma_start(t16[:, j2:j2 + 2, :],
                                            x16B[m][:, j2 * MT:(j2 + 2) * MT])
                else:
                    nc.scalar.dma_start(t16[:], x16B[m])
                return t16

            def load_x8(m):
                t8 = xpool.tile([KT, 2, MT], mybir.dt.float8e4,
                                name="xm8", tag="xm8")
                nc.scalar.dma_start(t8[:], x8B[m])
                return t8

            # j01 slices for all batch blocks first, then j23, j45, x8
            for m in range(GB):
                x16t[m] = xpool.tile([KT, K16T, MT], mybir.dt.bfloat16,
                                     name="xm16", tag="xm16")
            for j2 in range(0, K16T, 2):
                for m in range(GB):
                    nc.scalar.dma_start(
                        x16t[m][:, j2:j2 + 2, :],
                        x16B[m][:, j2 * MT:(j2 + 2) * MT])
            for m in range(GB):
                x8t[m] = load_x8(m)


            def drain_pair(m, g, accs2, split_last=False, phase_a=False):
                # descale + PSUM->SBUF bf16 for the n-pair (2g, 2g+1) of
                # block m; one 256 KB y DMA (alternating rings). Phase A
                # alternates scalar/gpsimd (the sync ring is full of w
                # descs and its 8-deep window would delay these, pinning
                # ot buffers and eventually stalling drains).
                ot = opool.tile([MT, 2 * NT], mybir.dt.bfloat16,
                                name="ot", tag="ot")
                if split_last:
                    # halve latency on the very last bank: DVE + ACT
                    # each do half of n-odd after n-even
                    nc.vector.tensor_scalar_mul(ot[:, 0:NT], accs2[0][:], INV)
                    nc.vector.tensor_scalar_mul(
                        ot[:, NT:NT + NT // 2], accs2[1][:, 0:NT // 2], INV)
                    nc.scalar.activation(
                        ot[:, NT + NT // 2:], accs2[1][:, NT // 2:],
                        mybir.ActivationFunctionType.Identity, scale=INV)
                    row0 = m * MT
                    nc.scalar.dma_start(
                        y[row0:row0 + MT, 2 * g * NT:(2 * g + 1) * NT],
                        ot[:, 0:NT])
                    nc.sync.dma_start(
                        y[row0:row0 + MT, (2 * g + 1) * NT:(2 * g + 2) * NT],
                        ot[:, NT:])
                else:
                    nc.vector.tensor_scalar_mul(ot[:, 0:NT], accs2[0][:], INV)
                    nc.scalar.activation(
                        ot[:, NT:], accs2[1][:],
                        mybir.ActivationFunctionType.Identity, scale=INV)
                    row0 = m * MT
                    if phase_a:
                        eng = nc.scalar if (m + g) % 2 == 0 else nc.gpsimd
                    else:
                        eng = nc.scalar if (m * 4 + g) % 2 == 0 else nc.sync
                    eng.dma_start(
                        y[row0:row0 + MT, 2 * g * NT:(2 * g + 2) * NT], ot[:])

            def mm16(acc, xt16, j, n, start, stop):
                nc.tensor.matmul(
                    acc[:], xt16[:, j, :], wt[j][:, n * NT:(n + 1) * NT],
                    start=start, stop=stop)

            def mm8(acc, xt8, n, start, stop):
                nc.tensor.matmul(
                    acc[:], xt8[:], w8t[:, :, n * NT:(n + 1) * NT],
                    start=start, stop=stop, perf_mode=DR)

            # ---- phase A: k-progressive groups over m-blocks 0..GB-1 ----
            # group g: (m 0..GB-1) x (n 2g, 2g+1); even g: bf16 js then
            # DR; odd g: DR then bf16 — adjacent DR sections run b2b.
            for g in range(NTILES // 2):
                accs = [[psum_pool.tile([MT, NT], mybir.dt.float32,
                                        name="acc", tag="acc")
                         for _ in range(2)] for _ in range(GB)]
                dr_first = (g % 2 == 1)
                if dr_first:
                    for m in range(GB):
                        for n in range(2):
                            mm8(accs[m][n], x8t[m], 2 * g + n, True, False)
                for j in range(K16T):
                    for m in range(GB):
                        for n in range(2):
                            mm16(accs[m][n], x16t[m], j, 2 * g + n,
                                 start=(j == 0 and not dr_first),
                                 stop=(j == K16T - 1 and dr_first))
                if not dr_first:
                    for m in range(GB):
                        for n in range(2):
                            mm8(accs[m][n], x8t[m], 2 * g + n, False, True)
                # prefetch steady-state x during the later groups
                if g == 2:
                    x16t[GB] = load_x16(GB)
                    x8t[GB] = load_x8(GB)
                if g == 3:
                    x16t[GB + 1] = load_x16(GB + 1)
                    x8t[GB + 1] = load_x8(GB + 1)
                for m in range(GB):
                    drain_pair(m, g, accs[m], phase_a=True)

            # ---- phase B: m = GB..MTILES-1, v1 steady-state loop ----
            for m in range(GB, MTILES):
                if m + 2 < MTILES:
                    x16t[m + 2] = load_x16(m + 2)
                    x8t[m + 2] = load_x8(m + 2)
                accs = [psum_pool.tile([MT, NT], mybir.dt.float32,
                                       name="acc", tag="acc")
                        for n in range(NTILES)]
                last_m = m == MTILES - 1
                if not last_m:
                    if m % 2 == 0:
                        for j in range(K16T):
                            for n in range(NTILES):
                                mm16(accs[n], x16t[m], j, n, j == 0, False)
                        for n in range(NTILES):
                            mm8(accs[n], x8t[m], n, False, True)
                    else:
                        for n in range(NTILES):
                            mm8(accs[n], x8t[m], n, True, False)
                        for j in range(K16T):
                            for n in range(NTILES):
                                mm16(accs[n], x16t[m], j, n, False,
                                     j == K16T - 1)
                    for g in range(NTILES // 2):
                        drain_pair(m, g, accs[2 * g:2 * g + 2])
                else:
                    # last block: n-outer/k-inner so drains + output DMA
                    # overlap the tail matmuls; split the final drain/DMA
                    for n in range(NTILES):
                        mm8(accs[n], x8t[m], n, True, False)
                        for j in range(K16T):
                            mm16(accs[n], x16t[m], j, n, False, j == K16T - 1)
                        if n % 2 == 1:
                            g = n // 2
                            drain_pair(m, g, accs[n - 1:n + 1],
                                       split_last=(n == NTILES - 1))

    nc.compile()
    return nc


def _install_neff_cache():
    """Disk-cache walrus NEFF compiles keyed on the BIR bytes."""
    if _CACHE.get("neff_cache_installed"):
        return
    _CACHE["neff_cache_installed"] = True
    import hashlib
    import os
    import shutil

    import concourse.bass2jax as bass2jax

    cache_dir = "/root/.neff_bir_cache"
    os.makedirs(cache_dir, exist_ok=True)
    orig = bass2jax.compile_bir_kernel

    def cached_compile(ant_bir_str, tmpdir, neff_name="file.neff", **kw):
        key = hashlib.sha256(
            ant_bir_str if isinstance(ant_bir_str, bytes)
            else ant_bir_str.encode()).hexdigest()
        hit = os.path.join(cache_dir, key + ".neff")
        dst = os.path.join(tmpdir, neff_name)
        if os.path.exists(hit):
            shutil.copyfile(hit, dst)
            return dst
        out = orig(ant_bir_str, tmpdir, neff_name=neff_name, **kw)
        try:
            shutil.copyfile(out, hit)
        except OSError:
            pass
        return out

    bass2jax.compile_bir_kernel = cached_compile


def _get_nc():
    if "nc" not in _CACHE:
        _install_neff_cache()
        _CACHE["nc"] = _build_nc()
    return _CACHE["nc"]


def kernel(input, weight, bias, num_experts_per_token):
    import ml_dtypes
    from concourse.bass_utils import run_bass_kernel_spmd

    input = np.ascontiguousarray(np.asarray(input, dtype=np.float32))
    weight = np.ascontiguousarray(np.asarray(weight, dtype=np.float32))
    bias = np.ascontiguousarray(np.asarray(bias, dtype=np.float32))
    counts = np.asarray(num_experts_per_token).astype(np.int64)
    offsets = np.concatenate([[0], np.cumsum(counts)]).astype(np.int64)

    if counts.max() > TOKC:
        # capacity overflow (never hit with balanced routing): numpy fallback
        outs = []
        for i in range(E):
            xi = input[offsets[i]:offsets[i + 1]]
            outs.append(xi @ weight[i].T + bias[i])
        return np.concatenate(outs, axis=0)

    bf16 = ml_dtypes.bfloat16
    e4m3 = ml_dtypes.float8_e4m3

    in_maps = []
    for i in range(E):
        wi = weight[i]                                  # [DOUT, DIN]
        xi = input[offsets[i]:offsets[i + 1]]           # [n_i, DIN]
        if xi.shape[0] < TOKC:
            xi = np.concatenate(
                [xi, np.zeros((TOKC - xi.shape[0], DIN), np.float32)], axis=0)

        # balanced scales with fixed product a*b = PSCALE so the drain
        # constant is shared across cores; clamp a into the range that
        # keeps BOTH fp8 operands in [0, F8MAX].
        mx = float(np.abs(xi[:, :KF8]).max())
        mw = float(np.abs(wi[:, :KF8]).max())
        if mx > 0 and mw > 0:
            a = float(np.sqrt(PSCALE * mw / mx))
            a = min(max(a, PSCALE * mw / (F8MAX * F8MARGIN)),
                    F8MAX * F8MARGIN / mx)
        else:
            a = 1.0
        b = PSCALE / a

        xs = xi * a
        # [m, kk, j, t] <- xs[m*128+t, (j+2)*128+kk]
        x16p = np.ascontiguousarray(
            xs[:, KF8:].reshape(MTILES, MT, K16T, KT)
            .transpose(0, 3, 2, 1)
            .reshape(MTILES, KT, K16T * MT)
            .astype(bf16))
        # [m, kk, i, t] <- xs[m*128+t, i*128+kk]
        x8p = np.ascontiguousarray(
            xs[:, :KF8].reshape(MTILES, MT, 2, KT)
            .transpose(0, 3, 2, 1)
            .reshape(MTILES, KT, 2 * MT)
            .astype(e4m3))
        ws = (wi * b).T                                 # [DIN, DOUT]
        # [j, kk, d] <- ws[(j+2)*128+kk, d]
        w16p = np.ascontiguousarray(
            ws[KF8:].reshape(K16T, KT, DOUT).astype(bf16))
        # [kk, i, d] <- ws[i*128+kk, d]
        w8p = np.ascontiguousarray(
            ws[:KF8].reshape(2, KT, DOUT)
            .transpose(1, 0, 2)
            .reshape(KT, 2 * DOUT)
            .astype(e4m3))
        in_maps.append({"x16B": x16p, "x8B": x8p,
                        "w16B": w16p, "w8B": w8p})

    nc = _get_nc()
    import os
    trace = bool(int(os.environ.get("KERNEL_TRACE", "0")))
    if trace:
        try:
            import axon_profile_shim
            axon_profile_shim.install()
            import antenv.axon_hooks  # noqa: F401
        except Exception:
            trace = False
    res = run_bass_kernel_spmd(nc, in_maps, core_ids=list(range(NCORES)),
                               trace=trace)
    _CACHE["last_result"] = res

    out = np.empty((T, DOUT), dtype=np.float32)
    pos = 0
    for i in range(E):
        n_i = int(counts[i])
        # bias is added here (host, fp32) rather than on-device
        out[pos:pos + n_i] = res.results[i]["y"][:n_i].astype(np.float32)
        out[pos:pos + n_i] += bias[i]
        pos += n_i
    return out
